# revision 54
# baseline (speedup 1.0000x reference)
"""Trainium2 Bass kernel for nn_MultiHeadContrastive (two-head contrastive loss).

Strategy (8 NeuronCores, two SPMD launches, no collectives):

  Launch 1 (MLP): rows of roi_feats are sorted by group
  (anchor / fg-low-iou / bg / ignore) on the host and sharded contiguously,
  1024 rows per core.  Each core computes both projection heads for its rows
  in fp8 (DoubleRow matmuls, 2x PE throughput; weights pre-scaled by 32 so
  fp8e4m3 resolution is used well, rescaled back in the output stage) and
  returns the raw (pre-normalization) embeddings as bf16.

  Host: gathers the 8 z shards, L2-normalizes rows in float64, casts bf16.

  Launch 2 (SIM): every core receives the full normalized key matrices
  zT_fg [64, 8192], zT_cls [128, 8192] (bf16) plus its private slice of
  anchor columns.  For each 128-anchor block it computes sim = anchors x keys
  via TensorE into PSUM per 2048-key group, ScalarE evaluates exp(dot/TAU)
  into SBUF bf16 tiles, and the idle DVE produces the per-anchor range sums
  the losses need with one fused tensor_tensor_reduce per group (plus one for
  the fg/non-fg boundary).  Because rows were sorted, the masked sums are
  plain prefix-range sums - no mask tensors exist at all.

  Host: subtracts self-similarity terms, computes the class-positive part of
  the SupCon loss from per-class sums of z (O(N*D)), applies logs/weights in
  float64, returns the 2-element loss vector.
"""

import math
import os

import numpy as np
import ml_dtypes

import concourse.bacc as bacc
import concourse.mybir as mybir
import concourse.tile as tile
from concourse.bass_utils import run_bass_kernel_spmd

N_CORES = 8
N, C = 8192, 1024
HID, DF, DC = 256, 64, 128
TAU = 0.2
EPS = 1e-8
EPS12 = 1e-12
IOU_THRESHOLD = 0.5

F32 = mybir.dt.float32
BF16 = mybir.dt.bfloat16
FP8 = mybir.dt.float8e4
ACT = mybir.ActivationFunctionType
AX = mybir.AxisListType
OP = mybir.AluOpType
DR = mybir.MatmulPerfMode.DoubleRow
LOG2E = 1.4426950408889634
LN2 = 0.6931471805599453
# cubic minimax-ish fit of 2^f on [0,1) for the DVE exp2 offload
EXP2_A3, EXP2_A2, EXP2_A1, EXP2_A0 = (
    0.07902014, 0.22412728, 0.69683757, 0.99981218)
# columns of fg group 1 whose exp runs on the DVE instead of ScalarE
OFF_L = 0  # DVE runs ~6.6ns/col on this pipeline - not competitive

NP_BF16 = ml_dtypes.bfloat16
NP_FP8 = ml_dtypes.float8_e4m3

# MLP input dtype: fp8 (DoubleRow, fastest) or bf16 fallback.
MLP_FP8 = os.environ.get("CC_MLP_FP8", "1") == "1"
WS = 32.0  # fp8 weight pre-scale (power of two)
ZS = 32.0  # fp8 z-output stage divisor (psum/ZS fits e4m3 range)

# Introspection for test.py.
LAST_RESULTS = []
LAST_TIMES = []

_NC_CACHE = {}


def _build_mlp_nc():
    """Launch 1: per-core MLP producing raw zT (bf16) for both heads."""
    R = N // N_CORES  # 1024 rows per core
    RB = 512
    NR = R // RB
    fp8 = MLP_FP8
    idt = FP8 if fp8 else BF16
    KP = 4 if fp8 else 8  # k-steps (256-row pairs for DoubleRow, else 128)

    nc = bacc.Bacc(trn_type="TRN2", num_devices=N_CORES, debug=False)
    xTd = nc.dram_tensor("xT", [C, R], idt, kind="ExternalInput")
    w1d = nc.dram_tensor("w1", [C, 2 * HID], idt, kind="ExternalInput")
    w2d = nc.dram_tensor("w2", [HID, DF + DC], idt, kind="ExternalInput")
    cst = nc.dram_tensor("cst", [128, 6], F32, kind="ExternalInput")
    odt = FP8 if fp8 else BF16
    zfd = nc.dram_tensor("zf", [DF, R], odt, kind="ExternalOutput")
    zcd = nc.dram_tensor("zc", [DC, R], odt, kind="ExternalOutput")

    with tile.TileContext(nc) as tc:
        with (
            tc.tile_pool(name="dat", bufs=1) as dat,
            tc.tile_pool(name="zb", bufs=2) as zb,
            tc.tile_pool(name="ps", bufs=1, space="PSUM") as ps,
        ):
            w1t = dat.tile([128, 8, 2 * HID], idt, tag="w1t")
            xt = dat.tile([128, 8, R], idt, tag="xt")
            # three large input DMAs: HWDGE issue overhead (625ns/DMA)
            # outweighs finer-grained streaming
            nc.sync.dma_start(
                out=w1t[:, :, :],
                in_=w1d[:, :].rearrange("(k p) n -> p k n", p=128),
            )
            nc.sync.dma_start(
                out=xt[:, 0:2, 0:RB],
                in_=xTd[0:256, 0:RB].rearrange("(k p) n -> p k n", p=128),
            )
            nc.sync.dma_start(
                out=xt[:, 2:8, 0:RB],
                in_=xTd[256:C, 0:RB].rearrange("(k p) n -> p k n", p=128),
            )
            nc.sync.dma_start(
                out=xt[:, :, RB:R],
                in_=xTd[:, RB:R].rearrange("(k p) n -> p k n", p=128),
            )
            cstt = dat.tile([128, 6], F32, tag="cst")
            nc.sync.dma_start(out=cstt[:, :], in_=cst[:, :])
            w2t = dat.tile([128, 2, DF + DC], idt, tag="w2t")
            nc.sync.dma_start(
                out=w2t[:, :, :],
                in_=w2d[:, :].rearrange("(k p) n -> p k n", p=128),
            )
            # preload the ACT Identity table and start the PE pstate-ramp
            # clock while DMAs stream
            wu = zb.tile([1, 8], F32, tag="wu")
            nc.vector.memset(wu[:, :], 0.0)
            nc.scalar.activation(out=wu[:, :], in_=wu[:, :], func=ACT.Identity,
                                 bias=0.0, scale=1.0)
            wub = zb.tile([1, 8], BF16, tag="wub")
            nc.vector.memset(wub[:, :], 0.0)
            wup = ps.tile([128, RB], F32, tag="p0_0", name="wup")
            nc.tensor.matmul(out=wup[0:1, 0:8], lhsT=wub[:, 0:1], rhs=wub[:, :],
                             start=True, stop=True)
            # second warmup gated on the w1 DMA keeps the pstate clock running
            # across the DMA head (a long PE idle resets the ramp)
            nc.tensor.matmul(out=wup[0:1, 8:16], lhsT=w1t[:, 0, 0:1],
                             rhs=w1t[:, 0, 0:8], start=True, stop=True)

            # hidden layer: 4 output blocks (head f kh0, f kh1, c kh0, c kh1)
            # x 2 row strips, accumulated over k
            hps = {}
            for hb in range(4):
                for r in range(NR):
                    hps[(hb, r)] = ps.tile(
                        [128, RB], F32, tag=f"p{hb}_{r}", name=f"hp{hb}{r}"
                    )
            for r in range(NR):
                for c in range(KP):
                    for hb in range(4):
                        hp = hps[(hb, r)]
                        if fp8:
                            nc.tensor.matmul(
                                out=hp[:, :],
                                lhsT=w1t[:, 2 * c:2 * c + 2, hb * 128:(hb + 1) * 128],
                                rhs=xt[:, 2 * c:2 * c + 2, r * RB:(r + 1) * RB],
                                start=(c == 0),
                                stop=(c == KP - 1),
                                perf_mode=DR,
                            )
                        else:
                            nc.tensor.matmul(
                                out=hp[:, :],
                                lhsT=w1t[:, c, hb * 128:(hb + 1) * 128],
                                rhs=xt[:, c, r * RB:(r + 1) * RB],
                                start=(c == 0),
                                stop=(c == KP - 1),
                            )
            # relu (+ pre-scaled bias): kh0 on DVE, kh1 on ScalarE in parallel
            hsb = {
                0: dat.tile([128, 2, R], idt, tag="hf", name="hf"),
                1: dat.tile([128, 2, R], idt, tag="hc", name="hc"),
            }
            for r in range(NR):
                for head in range(2):
                    for kh in range(2):
                        hb = head * 2 + kh
                        if kh == 0:
                            nc.vector.tensor_scalar(
                                out=hsb[head][:, kh, r * RB:(r + 1) * RB],
                                in0=hps[(hb, r)][:, :],
                                scalar1=cstt[:, hb:hb + 1],
                                scalar2=0.0,
                                op0=OP.add,
                                op1=OP.max,
                            )
                        else:
                            nc.scalar.activation(
                                out=hsb[head][:, kh, r * RB:(r + 1) * RB],
                                in_=hps[(hb, r)][:, :],
                                func=ACT.Relu,
                                bias=cstt[:, hb:hb + 1],
                                scale=1.0,
                            )
            # output layer: DMA straight from PSUM (fp32); the host applies
            # the 1/WS^2 rescale and output bias before normalizing (the
            # normalize is scale-invariant, so the rescale is exact)
            zst = {
                0: zb.tile([DF, R], odt, tag="zf", name="zf"),
                1: zb.tile([DC, R], odt, tag="zc", name="zc"),
            }
            zdim = {0: DF, 1: DC}
            zout = {0: zfd, 1: zcd}
            for r in range(NR):
                for head in range(2):
                    d = zdim[head]
                    cols = slice(0, DF) if head == 0 else slice(DF, DF + DC)
                    zp = ps.tile(
                        [128, RB], F32, tag=f"p{head * 2}_{r}", name=f"zp{head}{r}"
                    )
                    if fp8:
                        nc.tensor.matmul(
                            out=zp[:d, :],
                            lhsT=w2t[:, :, cols],
                            rhs=hsb[head][:, :, r * RB:(r + 1) * RB],
                            start=True,
                            stop=True,
                            perf_mode=DR,
                        )
                    else:
                        for kh in range(2):
                            nc.tensor.matmul(
                                out=zp[:d, :],
                                lhsT=w2t[:, kh, cols],
                                rhs=hsb[head][:, kh, r * RB:(r + 1) * RB],
                                start=(kh == 0),
                                stop=(kh == 1),
                            )
                    ss = 1.0 / ZS if fp8 else 1.0
                    if head == 0:
                        nc.vector.tensor_scalar(
                            out=zst[head][:, r * RB:(r + 1) * RB],
                            in0=zp[:d, :], scalar1=ss,
                            scalar2=0.0, op0=OP.mult, op1=OP.bypass,
                        )
                    else:
                        nc.scalar.mul(
                            out=zst[head][:, r * RB:(r + 1) * RB],
                            in_=zp[:d, :], mul=ss,
                        )
            for head in range(2):
                nc.sync.dma_start(out=zout[head][:, :], in_=zst[head][:, :])
    nc.compile()
    return nc


def _build_sim_nc(n_fg, n_valid, nblk, job_widths=()):
    """Launch 2: anchor-block sims, ScalarE exp, DVE fused range sums.

    stats[:, ab*NS + j] per main block: j=0 fg sum over all keys, j=1 cls
    sum over valid keys, j=2 fg tail sum (keys [n_fg, N)).  When n_jobs>0
    the leftover anchor blocks (those that would pad a 4th per-core block)
    are instead split into n_jobs host-packed [128x2048] jobs per core
    whose sums land in stats[:, nblk*NS + j]; zero-padded key columns
    contribute exactly exp(0)=1 and are subtracted on the host.  All sums
    include the self term.
    """
    A = nblk * 128
    G = 2048
    NGF = N // G
    NGC = (n_valid + G - 1) // G
    NS = 3 + (N + G - 1) // G  # 0 fg total, 1 cls chain, 2 fg tail, 3+ accums

    g_t = n_fg // G          # fg boundary group
    o_t = n_fg - g_t * G     # boundary offset within group
    tail_len = N - n_fg

    nc = bacc.Bacc(trn_type="TRN2", num_devices=N_CORES, debug=False)
    zfk = nc.dram_tensor("zfk", [DF, N], BF16, kind="ExternalInput")
    zck = nc.dram_tensor("zck", [DC, N], BF16, kind="ExternalInput")
    zfh = nc.dram_tensor("zfh", [DF, A + 512], BF16, kind="ExternalInput")
    zca = nc.dram_tensor("zca", [DC, A], BF16, kind="ExternalInput")
    n_jobs = len(job_widths)
    joff = [0]
    for w in job_widths:
        joff.append(joff[-1] + w)
    stats = nc.dram_tensor(
        "stats", [128, nblk * NS + n_jobs], F32, kind="ExternalOutput"
    )
    if n_jobs:
        zja = nc.dram_tensor("zja", [128, n_jobs * 128], BF16,
                             kind="ExternalInput")
        zjk = nc.dram_tensor("zjk", [128, joff[-1]], BF16,
                             kind="ExternalInput")

    F16 = mybir.dt.float16
    I16 = mybir.dt.int16

    def _make_dve_exp2(nc, xp):
        def _dve_exp2(p, eb, L):
            """eb[:, 0:L] = 2**p[:, 0:L] on the DVE (p holds log2-domain
            sims); frees ScalarE cycles on the launch's critical path."""
            ft = xp.tile([128, L], F16, tag="ft", name="ft")
            nc.vector.tensor_scalar(out=ft[:, :], in0=p[:, 0:L], scalar1=1.0,
                                    scalar2=0.0, op0=OP.mod, op1=OP.bypass)
            kt = xp.tile([128, L], I16, tag="kt", name="kt")
            nc.vector.tensor_tensor(out=kt[:, :], in0=p[:, 0:L], in1=ft[:, :],
                                    op=OP.subtract)
            nc.vector.tensor_scalar(out=kt[:, :], in0=kt[:, :], scalar1=15,
                                    scalar2=0.0, op0=OP.add, op1=OP.bypass)
            nc.vector.tensor_scalar(out=kt[:, :], in0=kt[:, :], scalar1=10,
                                    scalar2=0.0, op0=OP.logical_shift_left,
                                    op1=OP.bypass)
            p2k = kt[:, :].bitcast(F16)
            x1 = xp.tile([128, L], F16, tag="x1", name="x1")
            nc.vector.tensor_scalar(out=x1[:, :], in0=ft[:, :],
                                    scalar1=EXP2_A3, scalar2=0.0,
                                    op0=OP.mult, op1=OP.bypass)
            nc.vector.scalar_tensor_tensor(
                out=x1[:, :], in0=x1[:, :], scalar=EXP2_A2, in1=ft[:, :],
                op0=OP.add, op1=OP.mult)
            nc.vector.scalar_tensor_tensor(
                out=x1[:, :], in0=x1[:, :], scalar=EXP2_A1, in1=ft[:, :],
                op0=OP.add, op1=OP.mult)
            nc.vector.scalar_tensor_tensor(
                out=eb[:, 0:L], in0=x1[:, :], scalar=EXP2_A0, in1=p2k,
                op0=OP.add, op1=OP.mult)
        return _dve_exp2

    def fold_sum(scr, length, acc):
        """Fold scr[:, 0:length] pairwise in place, then reduce into acc."""
        while length > 128 and length % 2 == 0:
            h = length // 2
            nc.vector.tensor_tensor(
                out=scr[:, 0:h], in0=scr[:, 0:h], in1=scr[:, h:length], op=OP.add
            )
            length = h
        nc.vector.reduce_sum(out=acc, in_=scr[:, 0:length], axis=AX.X)

    with tile.TileContext(nc) as tc:
        with (
            tc.tile_pool(name="keys", bufs=1) as keys,
            tc.tile_pool(name="eb", bufs=4) as ebp,
            tc.tile_pool(name="scr", bufs=2) as scrp,
            tc.tile_pool(name="xp", bufs=2) as xpp,
            tc.tile_pool(name="ps", bufs=2, space="PSUM") as ps,
        ):
            _dve_exp2 = _make_dve_exp2(nc, xpp)
            # anchors + first key chunk share one tile so a single DMA
            # unblocks the first matmuls (HWDGE issues cost 625ns each)
            zfx_t = keys.tile([DF, A + N], BF16, tag="zfx")
            nc.sync.dma_start(out=zfx_t[:, 0:A + 512], in_=zfh[:, :])
            nc.sync.dma_start(out=zfx_t[:, A + 512:A + G], in_=zfk[:, 512:G])
            nc.sync.dma_start(out=zfx_t[:, A + G:A + 2 * G],
                              in_=zfk[:, G:2 * G])
            nc.sync.dma_start(out=zfx_t[:, A + 2 * G:A + N],
                              in_=zfk[:, 2 * G:N])
            zca_t = keys.tile([DC, A], BF16, tag="zca")
            nc.sync.dma_start(out=zca_t[:, :], in_=zca[:, :])
            zck_t = keys.tile([DC, N], BF16, tag="zck")
            nc.sync.dma_start(out=zck_t[:, 0:G], in_=zck[:, 0:G])
            nc.sync.dma_start(out=zck_t[:, G:N], in_=zck[:, G:N])
            if n_jobs:
                zja_t = keys.tile([128, n_jobs * 128], BF16, tag="zja")
                nc.sync.dma_start(out=zja_t[:, :], in_=zja[:, :])
                zjk_t = keys.tile([128, joff[-1]], BF16, tag="zjk")
                nc.sync.dma_start(out=zjk_t[:, :], in_=zjk[:, :])

            # preload the exp table and start the PE pstate-ramp clock
            # while DMAs stream
            wu = scrp.tile([1, 8], F32, tag="wu")
            nc.vector.memset(wu[:, :], 0.0)
            nc.scalar.activation(out=wu[:, :], in_=wu[:, :], func=ACT.Exp,
                                 scale=1.0)
            wub = scrp.tile([1, 8], BF16, tag="wub")
            nc.vector.memset(wub[:, :], 0.0)
            wup = ps.tile([128, G], F32, tag="ps", name="wup")
            nc.tensor.matmul(out=wup[0:1, 0:8], lhsT=wub[:, 0:1], rhs=wub[:, :],
                             start=True, stop=True)
            # re-warm gated on the anchor DMA so the first sim matmuls are
            # not stuck at a low pstate after the DMA-head idle
            nc.tensor.matmul(out=wup[0:1, 8:16], lhsT=zfx_t[:, 0:1],
                             rhs=zfx_t[:, 0:8], start=True, stop=True)

            st = keys.tile([128, nblk * NS + n_jobs], F32, tag="st")
            nc.vector.memset(st[:, :], 0.0)

            for ab in range(nblk):
                lf = zfx_t[:, ab * 128:(ab + 1) * 128]
                lc = zca_t[:, ab * 128:(ab + 1) * 128]
                for head in range(2):
                    lhs = lf if head == 0 else lc
                    zk = zfx_t[:, A:A + N] if head == 0 else zck_t
                    ng = NGF if head == 0 else NGC
                    ebs = []
                    for g in range(ng):
                        klim = G if head == 0 else min(G, n_valid - g * G)
                        # cls groups of the last block: sum on the ScalarE
                        # accumulator (exp in place in PSUM) so nothing on the
                        # DVE trails the final exp
                        act_accum = (
                            n_jobs == 0 and ab == nblk - 1 and head == 1
                        )
                        p = ps.tile([128, G], F32, tag="ps", name=f"ps{ab}{head}{g}")
                        for kk in range(math.ceil(klim / 512)):
                            nc.tensor.matmul(
                                out=p[:, kk * 512:(kk + 1) * 512],
                                lhsT=lhs,
                                rhs=zk[:, g * G + kk * 512:g * G + (kk + 1) * 512],
                                start=True,
                                stop=True,
                            )
                        if act_accum:
                            nc.scalar.activation(
                                out=p[:, 0:klim], in_=p[:, 0:klim], func=ACT.Exp,
                                scale=LN2,
                                accum_out=st[:, ab * NS + 3 + g:ab * NS + 4 + g],
                            )
                            continue
                        eb = ebp.tile([128, G], BF16, tag="eb", name=f"eb{ab}{head}{g}")
                        if klim < G:
                            # zero the invalid tail first; it is disjoint from
                            # the exp range and off the fold chain's wait path
                            nc.vector.memset(eb[:, klim:G], 0.0)
                        off = OFF_L if head == 0 and g == 1 else 0
                        nc.scalar.activation(
                            out=eb[:, off:klim], in_=p[:, off:klim],
                            func=ACT.Exp, scale=LN2,
                        )
                        if off:
                            _dve_exp2(p, eb, off)
                        ebs.append(eb)
                        if head == 0 and g == g_t and tail_len > 0:
                            # fg tail: sum over keys [n_fg, N) of this group
                            acc = st[:, ab * NS + 2:ab * NS + 3]
                            tl = G - o_t
                            if tl % 2 == 0:
                                th = tl // 2
                                scrt = scrp.tile([128, th], BF16, tag="scrt",
                                                 name=f"scrt{ab}")
                                nc.vector.tensor_tensor(
                                    out=scrt[:, :], in0=eb[:, o_t:o_t + th],
                                    in1=eb[:, o_t + th:G], op=OP.add,
                                )
                                fold_sum(scrt, th, acc)
                            else:
                                nc.vector.reduce_sum(out=acc, in_=eb[:, o_t:G],
                                                     axis=AX.X)
                    # head total: pair-sum the group tiles, fold, reduce
                    scr = scrp.tile([128, G], BF16, tag="scr", name=f"scr{ab}{head}")
                    if len(ebs) >= 2:
                        nc.vector.tensor_tensor(
                            out=scr[:, :], in0=ebs[0][:, :], in1=ebs[1][:, :],
                            op=OP.add,
                        )
                        for g in range(2, len(ebs)):
                            nc.vector.tensor_tensor(
                                out=scr[:, :], in0=ebs[g][:, :], in1=scr[:, :],
                                op=OP.add,
                            )
                        fold_sum(scr, G, st[:, ab * NS + head:ab * NS + head + 1])
                    elif len(ebs) == 1:
                        nc.vector.reduce_sum(
                            out=st[:, ab * NS + head:ab * NS + head + 1],
                            in_=ebs[0][:, :], axis=AX.X,
                        )
                # stats for this block are final once both heads' chains ran
                nc.sync.dma_start(
                    out=stats[:, ab * NS:(ab + 1) * NS],
                    in_=st[:, ab * NS:(ab + 1) * NS],
                )
            base = nblk * NS
            for j in range(n_jobs):
                W = job_widths[j]
                p = ps.tile([128, G], F32, tag="ps", name=f"psj{j}")
                for kk in range(W // 512):
                    nc.tensor.matmul(
                        out=p[:, kk * 512:(kk + 1) * 512],
                        lhsT=zja_t[:, j * 128:(j + 1) * 128],
                        rhs=zjk_t[:, joff[j] + kk * 512:joff[j] + (kk + 1) * 512],
                        start=True,
                        stop=True,
                    )
                if j == n_jobs - 1:
                    # last job sums on the ScalarE accumulator so nothing
                    # trails the final exp
                    nc.scalar.activation(
                        out=p[:, 0:W], in_=p[:, 0:W], func=ACT.Exp, scale=LN2,
                        accum_out=st[:, base + j:base + j + 1],
                    )
                else:
                    eb = ebp.tile([128, G], BF16, tag="eb", name=f"ebj{j}")
                    nc.scalar.activation(
                        out=eb[:, 0:W], in_=p[:, 0:W], func=ACT.Exp, scale=LN2
                    )
                    scr = scrp.tile([128, G], BF16, tag="scr", name=f"scrj{j}")
                    nc.vector.tensor_tensor(
                        out=scr[:, 0:W // 2], in0=eb[:, 0:W // 2],
                        in1=eb[:, W // 2:W], op=OP.add,
                    )
                    fold_sum(scr, W // 2, st[:, base + j:base + j + 1])
            if n_jobs:
                nc.sync.dma_start(
                    out=stats[:, base:base + n_jobs],
                    in_=st[:, base:base + n_jobs],
                )
    nc.compile()
    return nc


def _run(nc, in_maps, out_names):
    import time as _time

    if os.environ.get("CC_BASS_SIM") == "1":
        from concourse import bass_interp

        results = []
        for m in range(N_CORES):
            sim = bass_interp.CoreSim(nc, core_id=m)
            for k, v in in_maps[m].items():
                sim.tensor(k)[:] = v
            if nc.partition_id_tensor is not None:
                sim.tensor(nc.partition_id_tensor.name)[:] = np.array(
                    [[m]], dtype=np.uint32
                )
            sim.simulate()
            results.append(
                {name: np.array(sim.mem_tensor(name)) for name in out_names}
            )
        return results
    t0 = _time.monotonic()
    res = run_bass_kernel_spmd(nc, in_maps, core_ids=list(range(N_CORES)))
    LAST_TIMES.append(_time.monotonic() - t0)
    LAST_RESULTS.append(res)
    return res.results


def kernel(**inputs):
    global LAST_RESULTS, LAST_TIMES
    LAST_RESULTS = []
    LAST_TIMES = []

    roi = np.ascontiguousarray(np.asarray(inputs["roi_feats"], dtype=np.float32))
    labels = np.asarray(inputs["labels"]).astype(np.int64)
    ious = np.asarray(inputs["ious"], dtype=np.float32)
    w1f = np.asarray(inputs["w1f"], dtype=np.float32)
    b1f = np.asarray(inputs["b1f"], dtype=np.float32)
    w2f = np.asarray(inputs["w2f"], dtype=np.float32)
    b2f = np.asarray(inputs["b2f"], dtype=np.float32)
    w1c = np.asarray(inputs["w1c"], dtype=np.float32)
    b1c = np.asarray(inputs["b1c"], dtype=np.float32)
    w2c = np.asarray(inputs["w2c"], dtype=np.float32)
    b2c = np.asarray(inputs["b2c"], dtype=np.float32)
    assert roi.shape == (N, C)

    ign = labels == -1
    fg = (labels > 0) & ~ign
    bg = (labels == 0) & ~ign
    anc = fg & (ious > IOU_THRESHOLD)

    perm = np.concatenate(
        [
            np.where(anc)[0],
            np.where(fg & ~anc)[0],
            np.where(bg)[0],
            np.where(ign)[0],
        ]
    )
    n_A = int(anc.sum())
    n_fg = int(fg.sum())
    n_valid = n_fg + int(bg.sum())

    if n_A == 0:
        return np.zeros(2, dtype=np.float32)

    x_s = roi[perm]
    labels_s = labels[perm]
    ious_s = ious[perm].astype(np.float64)

    # ---------------- launch 1: MLP ----------------
    if "mlp" not in _NC_CACHE:
        _NC_CACHE["mlp"] = _build_mlp_nc()
    nc1 = _NC_CACHE["mlp"]
    np_idt = NP_FP8 if MLP_FP8 else NP_BF16
    ws = WS if MLP_FP8 else 1.0
    xT = np.ascontiguousarray(x_s.T).astype(np_idt)  # [C, N]
    R = N // N_CORES
    w1all = np.concatenate([w1f.T, w1c.T], axis=1) * ws  # [C, 512]
    w2all = np.concatenate([w2f.T, w2c.T], axis=1) * ws  # [HID, 192]
    cstv = np.zeros((128, 6), dtype=np.float32)
    cstv[:, 0] = b1f[0:128] * ws
    cstv[:, 1] = b1f[128:256] * ws
    cstv[:, 2] = b1c[0:128] * ws
    cstv[:, 3] = b1c[128:256] * ws
    cstv[0:DF, 4] = b2f
    cstv[:, 5] = b2c
    shared1 = {
        "w1": np.ascontiguousarray(w1all).astype(np_idt),
        "w2": np.ascontiguousarray(w2all).astype(np_idt),
        "cst": cstv,
    }
    in_maps1 = [
        {"xT": np.ascontiguousarray(xT[:, m * R:(m + 1) * R]), **shared1}
        for m in range(N_CORES)
    ]
    res1 = _run(nc1, in_maps1, ["zf", "zc"])

    zfT_raw = np.concatenate(
        [r["zf"].astype(np.float32) for r in res1], axis=1
    )  # [DF, N]
    zcT_raw = np.concatenate([r["zc"].astype(np.float32) for r in res1], axis=1)

    # ---------------- host: rescale+bias, normalize in float64 -------------
    ws2 = (WS * WS / ZS) if MLP_FP8 else 1.0

    def _normalize(zT_raw, b2):
        z = zT_raw.T.astype(np.float64) / ws2 + b2[None, :]  # [N, d]
        nrm = np.sqrt(np.sum(z * z, axis=1, keepdims=True))
        zn = z / np.maximum(nrm, EPS)
        return zn.astype(NP_BF16)

    zfn = _normalize(zfT_raw, b2f.astype(np.float64))  # [N, DF] bf16, sorted
    zcn = _normalize(zcT_raw, b2c.astype(np.float64))  # [N, DC]
    zfn64 = zfn.astype(np.float64)
    zcn64 = zcn.astype(np.float64)

    # ---------------- launch 2: sims ----------------
    G = 2048
    NS = 3 + (N + G - 1) // G
    nblk = n_A // (N_CORES * 128)  # full main blocks per core
    leftover = n_A - N_CORES * 128 * nblk
    if nblk == 0:
        nblk = max(1, math.ceil(math.ceil(n_A / N_CORES) / 128))
        leftover = 0
    # leftover anchors become host-packed jobs instead of a mostly-padded
    # extra block on every core; full-width (2048) denominator jobs fill
    # their own slots, the short fg-tail jobs share a narrow 1024 slot
    full_jobs = []
    tail_jobs = []
    if leftover > 0:
        a_base = N_CORES * 128 * nblk
        ngc = math.ceil(n_valid / G)
        tw = 1024 if (N - n_fg) <= 1024 else G
        for L in range(math.ceil(leftover / 128)):
            a0 = a_base + L * 128
            for g in range(N // G):
                full_jobs.append((0, "den", a0, g * G, (g + 1) * G, False))
            for g in range(ngc):
                full_jobs.append((1, "den", a0, g * G,
                                  min((g + 1) * G, n_valid), False))
            if n_fg < N:
                tail_jobs.append((0, "tail", a0, n_fg, N, False))
        while len(full_jobs) % N_CORES:
            h0, k0, a0, lo0, hi0, _ = full_jobs[0]
            full_jobs.append((h0, k0, a0, lo0, hi0, True))
        while len(tail_jobs) % N_CORES:
            h0, k0, a0, lo0, hi0, _ = tail_jobs[0]
            tail_jobs.append((h0, k0, a0, lo0, hi0, True))
    nf_slots = len(full_jobs) // N_CORES
    nt_slots = len(tail_jobs) // N_CORES
    # narrow tail slots go second-to-last: their DVE fold chains finish
    # during the final full-width job, whose ScalarE accumulator is the
    # last producer before the stats DMA
    if leftover > 0 and nf_slots > 0:
        job_widths = tuple(
            [G] * (nf_slots - 1) + [tw] * nt_slots + [G]
        )
    elif leftover > 0:
        job_widths = tuple([tw] * nt_slots)
    else:
        job_widths = ()
    n_jobs = len(job_widths)
    joff = [0]
    for w in job_widths:
        joff.append(joff[-1] + w)
    A_pc = nblk * 128
    sim_key = ("sim", n_fg, n_valid, nblk, job_widths)
    if sim_key not in _NC_CACHE:
        _NC_CACHE[sim_key] = _build_sim_nc(n_fg, n_valid, nblk, job_widths)
    nc2 = _NC_CACHE[sim_key]

    def _slot_job(m, j):
        if j < nf_slots - 1:
            return full_jobs[m * nf_slots + j]
        if j < nf_slots - 1 + nt_slots:
            return tail_jobs[m * nt_slots + (j - (nf_slots - 1))]
        return full_jobs[m * nf_slots + nf_slots - 1]

    zfkT = np.ascontiguousarray(zfn.T)  # [DF, N] bf16
    zckT = np.ascontiguousarray(zcn.T)  # [DC, N]
    asc = np.float32(LOG2E / TAU)
    zfaT = (zfkT.astype(np.float32) * asc).astype(NP_BF16)  # scaled anchors
    zcaT = (zckT.astype(np.float32) * asc).astype(NP_BF16)
    in_maps2 = []
    for m in range(N_CORES):
        idx = np.minimum(np.arange(m * A_pc, (m + 1) * A_pc), n_A - 1)
        maps = {
            "zfk": zfkT,
            "zck": zckT,
            "zfh": np.ascontiguousarray(
                np.concatenate([zfaT[:, idx], zfkT[:, 0:512]], axis=1)),
            "zca": np.ascontiguousarray(zcaT[:, idx]),
        }
        if n_jobs:
            ja = np.zeros((128, n_jobs * 128), dtype=NP_BF16)
            jk = np.zeros((128, joff[-1]), dtype=NP_BF16)
            for j in range(n_jobs):
                head, _, a0, lo, hi, _ = _slot_job(m, j)
                aidx = np.minimum(np.arange(a0, a0 + 128), n_A - 1)
                ln = hi - lo
                if head == 0:
                    ja[0:DF, j * 128:(j + 1) * 128] = zfaT[:, aidx]
                    jk[0:DF, joff[j]:joff[j] + ln] = zfkT[:, lo:hi]
                else:
                    ja[:, j * 128:(j + 1) * 128] = zcaT[:, aidx]
                    jk[:, joff[j]:joff[j] + ln] = zckT[:, lo:hi]
            maps["zja"] = ja
            maps["zjk"] = jk
        in_maps2.append(maps)
    res2 = _run(nc2, in_maps2, ["stats"])

    # main blocks: anchor a = core m, block ab, partition p
    sm = np.concatenate(
        [
            r["stats"][:, 0:nblk * NS].astype(np.float64)
            .reshape(128, nblk, NS).transpose(1, 0, 2)
            for r in res2
        ],
        axis=0,
    ).reshape(N_CORES * A_pc, NS)
    n_main = min(N_CORES * A_pc, n_A)
    denom_f = np.zeros(n_A)
    tail_f = np.zeros(n_A)
    denom_c = np.zeros(n_A)
    denom_f[:n_main] = sm[:n_main, 0]
    denom_c[:n_main] = sm[:n_main, 1] + sm[:n_main, 3:].sum(axis=1)
    tail_f[:n_main] = sm[:n_main, 2]
    # leftover jobs: each zero-padded key column contributed exp(0) = 1
    for m in range(N_CORES):
        js = res2[m]["stats"][:, nblk * NS:].astype(np.float64)
        for j in range(n_jobs):
            head, kind, a0, lo, hi, dup = _slot_job(m, j)
            if dup:
                continue
            rows = np.arange(128)
            valid = a0 + rows < n_A
            vals = js[rows[valid], j] - (job_widths[j] - (hi - lo))
            tgt = a0 + rows[valid]
            if head == 0 and kind == "den":
                denom_f[tgt] += vals
            elif head == 0:
                tail_f[tgt] += vals
            else:
                denom_c[tgt] += vals

    # ---------------- host: final losses in float64 ----------------
    zfa64 = zfn64[:n_A]
    zca64 = zcn64[:n_A]
    w_a = ious_s[:n_A]

    selfdot_f = np.sum(zfa64 * zfa64, axis=1)
    selfexp_f = np.exp(selfdot_f / TAU)
    selfdot_c = np.sum(zca64 * zca64, axis=1)
    selfexp_c = np.exp(selfdot_c / TAU)

    denom_all_f = denom_f
    numer_all_f = denom_f - tail_f
    denom_all_c = denom_c

    # fg/bg loss
    npos_fg = n_fg - 1
    if npos_fg > 0:
        denom = denom_all_f - selfexp_f
        numer = numer_all_f - selfexp_f
        li = -np.log((numer + EPS) / (denom + EPS))
        loss_fg = np.sum(li * w_a) / (np.sum(w_a) + EPS)
    else:
        loss_fg = 0.0

    # class supcon loss
    lab_valid = labels_s[:n_valid]
    cnt = np.bincount(lab_valid, minlength=21)
    S = np.zeros((21, DC), dtype=np.float64)
    np.add.at(S, lab_valid, zcn64[:n_valid])
    c_a = labels_s[:n_A]
    n_pos = (cnt[c_a] - 1).astype(np.float64)
    D = denom_all_c - selfexp_c
    denom_log = np.log(np.maximum(D, 1e-300))
    sum_pos = (np.einsum("nd,nd->n", zca64, S[c_a]) - selfdot_c) / TAU
    li_c = -(sum_pos - n_pos * denom_log) / np.maximum(n_pos, 1.0)
    valid_c = n_pos > 0
    num2 = np.sum(np.where(valid_c, li_c * w_a, 0.0))
    den2 = np.sum(np.where(valid_c, w_a, 0.0))
    loss_cls = num2 / (den2 + EPS12)

    return np.stack([loss_fg, loss_cls]).astype(np.float32)
